# revision 25
# baseline (speedup 1.0000x reference)
"""Trainium2 Bass kernel for nn_Att_cat_norm (gnn_message_passing).

reference:
    ln = LayerNorm(embeddings)                      # (B,N,D)
    sq = ln @ att_w[0,:D]; sk = ln @ att_w[0,D:]    # (B,N)
    logits = leaky_relu(sq[:,:,None] + sk[:,None,:] + att_b)
    alphas = softmax(logits, -1)[..., None]         # (B,N,N,1)
    value  = emb[:,:,None,:] * emb[:,None,:,:]      # (B,N,N,D)
    return alphas, value

Sharding: 8 cores; core c handles batch b=c//2, query rows i in
[i0, i0+256), i0=(c%2)*256.  Each core writes value rows (256,512,128)
= 64MB + alphas (256,512); output-DMA bound (~185us/core roofline).

Per-core device program (identical on all cores; per-core data arrives
via the inputs):
  value: partition dim = own query row (two blocks of 128).  Per group
  of 16 j-rows: PE broadcasts those rows to all 128 partitions with an
  fp32 ones-weight matmul (bit-exact: the fp32 two-pass split applies
  to the stationary operand, and 1.0 splits exactly), then one DVE
  tensor_mul (FD=2048) with a stride-0 view of the own-row tile, then
  one 1MB DMA whose descriptors are 8KB contiguous per partition.
  scores: LayerNorm stats via fused scalar_tensor_tensor + accum_out
  (var = E[(x-mu)x]); sq/sk as dot(x-mu, gamma*w)*rstd + dot(beta, w);
  sk transposed to a row via a DRAM-scratch round trip; sk broadcast
  via PE; logits = sk_bcast + sq + bias in one dual-scalar DVE op;
  Lrelu on ACT; exp + row-sum in one ACT op; normalize on DVE.
  The score pipeline is interleaved between early value groups so the
  output DMA stream never starves while the serial LN chain runs.
"""
import sys

if "/opt/trn_rl_repo" not in sys.path:
    sys.path.insert(0, "/opt/trn_rl_repo")

import numpy as np

B, N, D = 4, 512, 128
NI = N // 2          # own query rows per core
NCORES = 8
JG = 8               # j rows per value group
NJG = N // JG        # 64 value j-groups
NT_ALL = N // 128    # 4 token tiles of emb_all
NT_OWN = NI // 128   # 2 token tiles of emb_own
LEAKY_SLOPE = 0.01
LN_EPS = 1e-5
# params input layout: [att_w (2D) | att_b (1) | gamma (D) | beta (D)]
NPARAMS = 2 * D + 1 + D + D

_CACHE = {}


def _build_program(reps=1):
    import concourse.tile as tile
    from concourse import bacc, mybir

    F32 = mybir.dt.float32
    AF = mybir.ActivationFunctionType
    ALU = mybir.AluOpType
    AX = mybir.AxisListType

    nc = bacc.Bacc("TRN2", target_bir_lowering=False, debug=False,
                   num_devices=NCORES)

    emb_all = nc.dram_tensor("emb_all", (N, D), F32, kind="ExternalInput")
    emb_own = nc.dram_tensor("emb_own", (NI, D), F32, kind="ExternalInput")
    emb_flat = nc.dram_tensor("emb_flat", (1, N * D), F32, kind="ExternalInput")
    params = nc.dram_tensor("params", (1, NPARAMS), F32, kind="ExternalInput")

    value_out = nc.dram_tensor("value_out", (NI, N, D), F32, kind="ExternalOutput")
    alphas_out = nc.dram_tensor("alphas_out", (NI, N), F32, kind="ExternalOutput")

    with tile.TileContext(nc) as tc:
        with tc.tile_pool(name="consts", bufs=1) as consts, \
             tc.tile_pool(name="temps", bufs=3) as temps, \
             tc.tile_pool(name="soft", bufs=2) as soft, \
             tc.tile_pool(name="valp", bufs=10) as valp, \
             tc.tile_pool(name="pbc", bufs=4, space="PSUM") as pbc, \
             tc.tile_pool(name="dscr", bufs=1, space="DRAM") as dscr:

            # ---------- constants / inputs ----------
            ones_all = consts.tile([65, 128], F32)
            nc.vector.memset(ones_all, 1.0)
            ones_col = ones_all[0:1, :]
            eps_vec = consts.tile([128, 1], F32)
            nc.vector.memset(eps_vec, LN_EPS)

            # flat emb_all rows for the PE-broadcast rhs: 8-row groups
            # (4KB each) packed into partitions 0/32/64 (matmul base-
            # partition constraint), 22/22/20 groups per partition.
            prm = consts.tile([1, NPARAMS], F32)
            nc.sync.dma_start(out=prm, in_=params[:, :])
            emb_flat_sb = consts.tile([65, 22 * JG * D], F32)
            for pi, (g0, g1) in enumerate(((0, 22), (22, 44), (44, 64))):
                nc.sync.dma_start(
                    out=emb_flat_sb[32 * pi:32 * pi + 1, 0:(g1 - g0) * JG * D],
                    in_=emb_flat[0:1, g0 * JG * D:g1 * JG * D])
            emb_own_t = consts.tile([128, NT_OWN, D], F32)
            nc.sync.dma_start(
                out=emb_own_t,
                in_=emb_own[:, :].rearrange("(tt tp) d -> tp tt d", tp=128))
            emb_all_t = consts.tile([128, NT_ALL, D], F32)
            nc.sync.dma_start(
                out=emb_all_t,
                in_=emb_all[:, :].rearrange("(tt tp) d -> tp tt d", tp=128))
            w_q = prm[:, 0:D]
            w_k = prm[:, D:2 * D]
            att_b = prm[:, 2 * D:2 * D + 1]
            gamma = prm[:, 2 * D + 1:3 * D + 1]
            beta = prm[:, 3 * D + 1:4 * D + 1]

            # ---------- value group emitter ----------
            def emit_value_group(g):
                part = 32 * (g // 22)
                off = (g % 22) * JG * D
                bc_ps = pbc.tile([128, JG * D], F32, tag="bc")
                for q in range(JG * D // 512):
                    nc.tensor.matmul(
                        bc_ps[:, q * 512:(q + 1) * 512],
                        lhsT=ones_all[part:part + 1, :],
                        rhs=emb_flat_sb[part:part + 1,
                                        off + q * 512:off + (q + 1) * 512],
                        start=True, stop=True)
                for k in range(NT_OWN):
                    val = valp.tile([128, JG * D], F32, tag="val")
                    in0 = emb_own_t[:, k, :].unsqueeze(1).broadcast_to(
                        (128, JG, D))
                    in1 = bc_ps[:, :].rearrange("p (j d) -> p j d", j=JG)
                    outv = val[:, :].rearrange("p (j d) -> p j d", j=JG)
                    nc.vector.tensor_mul(out=outv, in0=in0, in1=in1)
                    nc.sync.dma_start(
                        out=value_out[k * 128:(k + 1) * 128,
                                      g * JG:(g + 1) * JG, :],
                        in_=outv)

            # ---------- score-pipeline emitters ----------
            state = {}

            def emit_setup():
                # wq' = gamma*wq, wk' = gamma*wk; bias = att_b+beta.wq+beta.wk
                wqp = consts.tile([1, D], F32, tag="wqp")
                nc.vector.tensor_mul(out=wqp, in0=w_q, in1=gamma)
                wkp = consts.tile([1, D], F32, tag="wkp")
                nc.vector.tensor_mul(out=wkp, in0=w_k, in1=gamma)
                scrq = temps.tile([1, D], F32, tag="scr1")
                cq = consts.tile([1, 1], F32, tag="cq")
                nc.vector.scalar_tensor_tensor(
                    out=scrq, in0=w_q, scalar=0.0, in1=beta,
                    op0=ALU.add, op1=ALU.mult, accum_out=cq)
                scrk = temps.tile([1, D], F32, tag="scr1")
                ck = consts.tile([1, 1], F32, tag="ck")
                nc.vector.scalar_tensor_tensor(
                    out=scrk, in0=w_k, scalar=0.0, in1=beta,
                    op0=ALU.add, op1=ALU.mult, accum_out=ck)
                bias_tot = consts.tile([1, 1], F32, tag="bias_tot")
                nc.vector.tensor_add(out=bias_tot, in0=cq, in1=ck)
                nc.vector.tensor_add(out=bias_tot, in0=bias_tot, in1=att_b)

                bvec_ps = pbc.tile([128, 1], F32, tag="bc")
                nc.tensor.matmul(bvec_ps, lhsT=ones_col, rhs=bias_tot,
                                 start=True, stop=True)
                bias_vec = consts.tile([128, 1], F32, tag="bias_vec")
                nc.scalar.copy(out=bias_vec, in_=bvec_ps)
                wqb_ps = pbc.tile([128, D], F32, tag="bc")
                nc.tensor.matmul(wqb_ps, lhsT=ones_col, rhs=wqp,
                                 start=True, stop=True)
                wqb = consts.tile([128, D], F32, tag="wqb")
                nc.scalar.copy(out=wqb, in_=wqb_ps)
                wkb_ps = pbc.tile([128, D], F32, tag="bc")
                nc.tensor.matmul(wkb_ps, lhsT=ones_col, rhs=wkp,
                                 start=True, stop=True)
                wkb = consts.tile([128, D], F32, tag="wkb")
                nc.scalar.copy(out=wkb, in_=wkb_ps)
                state.update(bias_vec=bias_vec, wqb=wqb, wkb=wkb)
                state["sq_colv"] = consts.tile([128, NT_OWN], F32, tag="sq_colv", name="sq_colv")
                state["sk_colv"] = consts.tile([128, NT_ALL], F32, tag="sk_colv", name="sk_colv")

            def emit_ln_tile(which, tt):
                # LayerNorm stats + raw q/k score for one 128-token tile.
                if which == "all":
                    x, w_ps, dst = (emb_all_t[:, tt, :], state["wkb"],
                                    state["sk_colv"][:, tt:tt + 1])
                else:
                    x, w_ps, dst = (emb_own_t[:, tt, :], state["wqb"],
                                    state["sq_colv"][:, tt:tt + 1])
                musum = temps.tile([128, 1], F32, tag="musum")
                nc.vector.tensor_reduce(out=musum, in_=x, axis=AX.X, op=ALU.add)
                negmu = temps.tile([128, 1], F32, tag="negmu")
                nc.vector.tensor_scalar_mul(out=negmu, in0=musum,
                                            scalar1=-1.0 / D)
                scr = temps.tile([128, D], F32, tag="scr")
                varsum = temps.tile([128, 1], F32, tag="varsum")
                nc.vector.scalar_tensor_tensor(
                    out=scr, in0=x, scalar=negmu[:, 0:1], in1=x,
                    op0=ALU.add, op1=ALU.mult, accum_out=varsum)
                stdv = temps.tile([128, 1], F32, tag="stdv")
                nc.scalar.activation(out=stdv, in_=varsum, func=AF.Sqrt,
                                     bias=eps_vec[:, 0:1], scale=1.0 / D)
                rstd = temps.tile([128, 1], F32, tag="rstd")
                nc.vector.reciprocal(rstd, stdv)
                scr2 = temps.tile([128, D], F32, tag="scr2")
                ssum = temps.tile([128, 1], F32, tag="ssum")
                nc.vector.scalar_tensor_tensor(
                    out=scr2, in0=x, scalar=negmu[:, 0:1], in1=w_ps,
                    op0=ALU.add, op1=ALU.mult, accum_out=ssum)
                nc.vector.tensor_mul(out=dst, in0=ssum, in1=rstd)

            def emit_sk_row():
                # transpose sk (128,NT_ALL) -> flat row in DRAM scratch, then
                # partition-broadcast it back to all 128 partitions.  Uses the
                # gpsimd (SWDGE) queue so the SP queue stays pure value DMAs.
                sk_dram = dscr.tile([NT_ALL, 128], F32, tag="sk_dram")
                nc.gpsimd.dma_start(out=sk_dram[:, :].transpose([1, 0]),
                                    in_=state["sk_colv"][:, :])
                sk_bc = consts.tile([128, N], F32, tag="sk_bc")
                sk_flat = sk_dram[:, :].rearrange("tt tp -> (tt tp)").unsqueeze(0)
                import concourse.bass as bass
                sk_bcast_src = bass.AP(
                    tensor=sk_flat.tensor, offset=sk_flat.offset,
                    ap=[[0, 128]] + list(sk_flat.ap)[1:])
                nc.gpsimd.dma_start(out=sk_bc, in_=sk_bcast_src)
                state["sk_bc"] = sk_bc

            def emit_softmax(k):
                logits = soft.tile([128, N], F32, tag="logits")
                nc.vector.tensor_scalar(
                    out=logits, in0=state["sk_bc"],
                    scalar1=state["sq_colv"][:, k:k + 1],
                    scalar2=state["bias_vec"][:, 0:1],
                    op0=ALU.add, op1=ALU.add)
                leak = soft.tile([128, N], F32, tag="leak")
                nc.vector.scalar_tensor_tensor(
                    out=leak, in0=logits, scalar=LEAKY_SLOPE, in1=logits,
                    op0=ALU.mult, op1=ALU.max)
                negmax = temps.tile([128, 1], F32, tag="negmax")
                nc.vector.tensor_reduce(out=negmax, in_=leak, axis=AX.X,
                                        op=ALU.max, negate=True)
                ex = soft.tile([128, N], F32, tag="ex")
                sume = temps.tile([128, 1], F32, tag="sume")
                nc.scalar.activation(out=ex, in_=leak, func=AF.Exp,
                                     bias=negmax[:, 0:1], scale=1.0,
                                     accum_out=sume[:, 0:1])
                rs = temps.tile([128, 1], F32, tag="rs")
                nc.vector.reciprocal(rs, sume)
                alph = soft.tile([128, N], F32, tag="alph")
                nc.vector.tensor_scalar_mul(out=alph, in0=ex, scalar1=rs[:, 0:1])
                nc.gpsimd.dma_start(out=alphas_out[k * 128:(k + 1) * 128, :],
                                    in_=alph)

            # ---------- interleaved emission ----------
            # Score-pipeline steps dripped between value groups so the DVE
            # never starves the output-DMA stream.
            # PE warm-up: ~3.5us of continuous small matmuls so the tensor
            # engine clock is at full speed when the first real broadcasts
            # arrive (PE ramps to 2.4GHz only after ~3us of sustained work).
            warm_ps = pbc.tile([128, 64], F32, tag="bc")
            NWARM = 16
            for w in range(NWARM):
                nc.tensor.matmul(warm_ps, lhsT=ones_col,
                                 rhs=ones_all[0:1, 0:64],
                                 start=(w == 0), stop=(w == NWARM - 1),
                                 skip_group_check=True)
            warm_scr = temps.tile([128, 1], F32, tag="warm_scr")
            nc.vector.tensor_reduce(out=warm_scr, in_=warm_ps, axis=AX.X,
                                    op=ALU.add)

            # body emitted `reps` times (reps>1 builds a timing NEFF whose
            # wall-clock slope isolates per-iteration HW time)
            for _rep in range(reps):
                emit_setup()
                steps = {
                    4: lambda: emit_ln_tile("all", 0),
                    6: lambda: emit_ln_tile("all", 1),
                    8: lambda: emit_ln_tile("all", 2),
                    10: lambda: emit_ln_tile("all", 3),
                    12: lambda: emit_ln_tile("own", 0),
                    14: lambda: emit_ln_tile("own", 1),
                    16: emit_sk_row,
                    24: lambda: emit_softmax(0),
                    28: lambda: emit_softmax(1),
                }
                for g in range(NJG):
                    emit_value_group(g)
                    if g in steps:
                        steps[g]()
                state.clear()

    nc.compile()
    return nc


def _get_program():
    if "nc" not in _CACHE:
        _CACHE["nc"] = _build_program()
    return _CACHE["nc"]


def _make_in_maps(embeddings, att_w, att_b, ln_gamma, ln_beta):
    emb = np.ascontiguousarray(embeddings, dtype=np.float32)
    prm = np.concatenate([
        np.asarray(att_w, np.float32).reshape(-1),
        np.asarray(att_b, np.float32).reshape(-1),
        np.asarray(ln_gamma, np.float32).reshape(-1),
        np.asarray(ln_beta, np.float32).reshape(-1),
    ]).reshape(1, NPARAMS)
    in_maps = []
    for c in range(NCORES):
        b, i0 = c // 2, (c % 2) * NI
        eb = np.ascontiguousarray(emb[b])
        in_maps.append({
            "emb_all": eb,
            "emb_own": np.ascontiguousarray(emb[b, i0:i0 + NI]),
            "emb_flat": eb.reshape(1, N * D),
            "params": prm,
        })
    return in_maps


def kernel(embeddings, att_w, att_b, ln_gamma, ln_beta):
    from concourse.bass_utils import run_bass_kernel_spmd

    nc = _get_program()
    in_maps = _make_in_maps(embeddings, att_w, att_b, ln_gamma, ln_beta)
    res = run_bass_kernel_spmd(nc, in_maps, core_ids=list(range(NCORES)))
    _CACHE["last_results"] = res

    alphas = np.empty((B, N, N, 1), np.float32)
    value = np.empty((B, N, N, D), np.float32)
    for c in range(NCORES):
        b, i0 = c // 2, (c % 2) * NI
        r = res.results[c]
        alphas[b, i0:i0 + NI, :, 0] = r["alphas_out"]
        value[b, i0:i0 + NI] = r["value_out"]
    return alphas, value


# revision 28
# speedup vs baseline: 90.2720x; 90.2720x over previous
"""Trainium2 Bass kernel for nn_Att_cat_norm (gnn_message_passing).

reference:
    ln = LayerNorm(embeddings)                      # (B,N,D)
    sq = ln @ att_w[0,:D]; sk = ln @ att_w[0,D:]    # (B,N)
    logits = leaky_relu(sq[:,:,None] + sk[:,None,:] + att_b)
    alphas = softmax(logits, -1)[..., None]         # (B,N,N,1)
    value  = emb[:,:,None,:] * emb[:,None,:,:]      # (B,N,N,D)
    return alphas, value

Sharding: 8 cores; core c handles batch b=c//2, query rows i in
[i0, i0+256), i0=(c%2)*256.  Each core writes value rows (256,512,128)
= 64MB + alphas (256,512); output-DMA bound (~185us/core roofline).

Per-core device program (identical on all cores; per-core data arrives
via the inputs):
  value: partition dim = own query row (two blocks of 128).  Per group
  of 16 j-rows: PE broadcasts those rows to all 128 partitions with an
  fp32 ones-weight matmul (bit-exact: the fp32 two-pass split applies
  to the stationary operand, and 1.0 splits exactly), then one DVE
  tensor_mul (FD=2048) with a stride-0 view of the own-row tile, then
  one 1MB DMA whose descriptors are 8KB contiguous per partition.
  scores: LayerNorm stats via fused scalar_tensor_tensor + accum_out
  (var = E[(x-mu)x]); sq/sk as dot(x-mu, gamma*w)*rstd + dot(beta, w);
  sk transposed to a row via a DRAM-scratch round trip; sk broadcast
  via PE; logits = sk_bcast + sq + bias in one dual-scalar DVE op;
  Lrelu on ACT; exp + row-sum in one ACT op; normalize on DVE.
  The score pipeline is interleaved between early value groups so the
  output DMA stream never starves while the serial LN chain runs.
"""
import sys

if "/opt/trn_rl_repo" not in sys.path:
    sys.path.insert(0, "/opt/trn_rl_repo")

import numpy as np

B, N, D = 4, 512, 128
NI = N // 2          # own query rows per core
NCORES = 8
JG = 8               # j rows per value group
NJG = N // JG        # 64 value j-groups
NT_ALL = N // 128    # 4 token tiles of emb_all
NT_OWN = NI // 128   # 2 token tiles of emb_own
LEAKY_SLOPE = 0.01
LN_EPS = 1e-5
# params input layout: [att_w (2D) | att_b (1) | gamma (D) | beta (D)]
NPARAMS = 2 * D + 1 + D + D

_CACHE = {}


def _build_program(reps=1):
    import concourse.tile as tile
    from concourse import bacc, mybir

    F32 = mybir.dt.float32
    AF = mybir.ActivationFunctionType
    ALU = mybir.AluOpType
    AX = mybir.AxisListType

    nc = bacc.Bacc("TRN2", target_bir_lowering=False, debug=False,
                   num_devices=NCORES)

    emb_all = nc.dram_tensor("emb_all", (N, D), F32, kind="ExternalInput")
    emb_own = nc.dram_tensor("emb_own", (NI, D), F32, kind="ExternalInput")
    emb_flat = nc.dram_tensor("emb_flat", (1, N * D), F32, kind="ExternalInput")
    params = nc.dram_tensor("params", (1, NPARAMS), F32, kind="ExternalInput")

    value_out = nc.dram_tensor("value_out", (NI, N, D), F32, kind="ExternalOutput")
    alphas_out = nc.dram_tensor("alphas_out", (NI, N), F32, kind="ExternalOutput")

    with tile.TileContext(nc) as tc:
        with tc.tile_pool(name="consts", bufs=1) as consts, \
             tc.tile_pool(name="temps", bufs=3) as temps, \
             tc.tile_pool(name="soft", bufs=2) as soft, \
             tc.tile_pool(name="valp", bufs=10) as valp, \
             tc.tile_pool(name="pbc", bufs=4, space="PSUM") as pbc, \
             tc.tile_pool(name="dscr", bufs=1, space="DRAM") as dscr:

            # ---------- constants / inputs ----------
            ones_all = consts.tile([65, 128], F32)
            nc.vector.memset(ones_all, 1.0)
            ones_col = ones_all[0:1, :]
            eps_vec = consts.tile([128, 1], F32)
            nc.vector.memset(eps_vec, LN_EPS)

            # flat emb_all rows for the PE-broadcast rhs: 8-row groups
            # (4KB each) packed into partitions 0/32/64 (matmul base-
            # partition constraint), 22/22/20 groups per partition.
            prm = consts.tile([1, NPARAMS], F32)
            nc.sync.dma_start(out=prm, in_=params[:, :])
            emb_flat_sb = consts.tile([65, 22 * JG * D], F32)
            for pi, (g0, g1) in enumerate(((0, 22), (22, 44), (44, 64))):
                nc.sync.dma_start(
                    out=emb_flat_sb[32 * pi:32 * pi + 1, 0:(g1 - g0) * JG * D],
                    in_=emb_flat[0:1, g0 * JG * D:g1 * JG * D])
            emb_own_t = consts.tile([128, NT_OWN, D], F32)
            nc.sync.dma_start(
                out=emb_own_t,
                in_=emb_own[:, :].rearrange("(tt tp) d -> tp tt d", tp=128))
            emb_all_t = consts.tile([128, NT_ALL, D], F32)
            nc.sync.dma_start(
                out=emb_all_t,
                in_=emb_all[:, :].rearrange("(tt tp) d -> tp tt d", tp=128))
            w_q = prm[:, 0:D]
            w_k = prm[:, D:2 * D]
            att_b = prm[:, 2 * D:2 * D + 1]
            gamma = prm[:, 2 * D + 1:3 * D + 1]
            beta = prm[:, 3 * D + 1:4 * D + 1]

            # ---------- value segment emitter ----------
            # j rows [j0, j0+nrows): PE-broadcast them to all partitions,
            # multiply with the own-row tile, DMA out.  nrows <= JG.
            def emit_value_seg(j0, nrows):
                part = 32 * (j0 // 176)
                off = (j0 - 176 * (j0 // 176)) * D
                bc_ps = pbc.tile([128, JG * D], F32, tag="bc")
                nmm = (nrows * D + 511) // 512
                for q in range(nmm):
                    f0, f1 = q * 512, min((q + 1) * 512, nrows * D)
                    nc.tensor.matmul(
                        bc_ps[:, f0:f1],
                        lhsT=ones_all[part:part + 1, :],
                        rhs=emb_flat_sb[part:part + 1, off + f0:off + f1],
                        start=True, stop=True)
                for k in range(NT_OWN):
                    val = valp.tile([128, JG * D], F32, tag="val")
                    in0 = emb_own_t[:, k, :].unsqueeze(1).broadcast_to(
                        (128, nrows, D))
                    in1 = bc_ps[:, 0:nrows * D].rearrange(
                        "p (j d) -> p j d", j=nrows)
                    outv = val[:, 0:nrows * D].rearrange(
                        "p (j d) -> p j d", j=nrows)
                    nc.vector.tensor_mul(out=outv, in0=in0, in1=in1)
                    nc.sync.dma_start(
                        out=value_out[k * 128:(k + 1) * 128,
                                      j0:j0 + nrows, :],
                        in_=outv)

            # ---------- score-pipeline emitters ----------
            state = {}

            def emit_setup():
                # wq' = gamma*wq, wk' = gamma*wk; bias = att_b+beta.wq+beta.wk
                wqp = consts.tile([1, D], F32, tag="wqp")
                nc.vector.tensor_mul(out=wqp, in0=w_q, in1=gamma)
                wkp = consts.tile([1, D], F32, tag="wkp")
                nc.vector.tensor_mul(out=wkp, in0=w_k, in1=gamma)
                scrq = temps.tile([1, D], F32, tag="scr1")
                cq = consts.tile([1, 1], F32, tag="cq")
                nc.vector.scalar_tensor_tensor(
                    out=scrq, in0=w_q, scalar=0.0, in1=beta,
                    op0=ALU.add, op1=ALU.mult, accum_out=cq)
                scrk = temps.tile([1, D], F32, tag="scr1")
                ck = consts.tile([1, 1], F32, tag="ck")
                nc.vector.scalar_tensor_tensor(
                    out=scrk, in0=w_k, scalar=0.0, in1=beta,
                    op0=ALU.add, op1=ALU.mult, accum_out=ck)
                bias_tot = consts.tile([1, 1], F32, tag="bias_tot")
                nc.vector.tensor_add(out=bias_tot, in0=cq, in1=ck)
                nc.vector.tensor_add(out=bias_tot, in0=bias_tot, in1=att_b)

                bvec_ps = pbc.tile([128, 1], F32, tag="bc")
                nc.tensor.matmul(bvec_ps, lhsT=ones_col, rhs=bias_tot,
                                 start=True, stop=True)
                bias_vec = consts.tile([128, 1], F32, tag="bias_vec")
                nc.scalar.copy(out=bias_vec, in_=bvec_ps)
                wqb_ps = pbc.tile([128, D], F32, tag="bc")
                nc.tensor.matmul(wqb_ps, lhsT=ones_col, rhs=wqp,
                                 start=True, stop=True)
                wqb = consts.tile([128, D], F32, tag="wqb")
                nc.scalar.copy(out=wqb, in_=wqb_ps)
                wkb_ps = pbc.tile([128, D], F32, tag="bc")
                nc.tensor.matmul(wkb_ps, lhsT=ones_col, rhs=wkp,
                                 start=True, stop=True)
                wkb = consts.tile([128, D], F32, tag="wkb")
                nc.scalar.copy(out=wkb, in_=wkb_ps)
                state.update(bias_vec=bias_vec, wqb=wqb, wkb=wkb)
                state["sq_colv"] = consts.tile([128, NT_OWN], F32, tag="sq_colv", name="sq_colv")
                state["sk_colv"] = consts.tile([128, NT_ALL], F32, tag="sk_colv", name="sk_colv")

            def emit_ln_tile(which, tt):
                # LayerNorm stats + raw q/k score for one 128-token tile.
                if which == "all":
                    x, w_ps, dst = (emb_all_t[:, tt, :], state["wkb"],
                                    state["sk_colv"][:, tt:tt + 1])
                else:
                    x, w_ps, dst = (emb_own_t[:, tt, :], state["wqb"],
                                    state["sq_colv"][:, tt:tt + 1])
                musum = temps.tile([128, 1], F32, tag="musum")
                nc.vector.tensor_reduce(out=musum, in_=x, axis=AX.X, op=ALU.add)
                negmu = temps.tile([128, 1], F32, tag="negmu")
                nc.vector.tensor_scalar_mul(out=negmu, in0=musum,
                                            scalar1=-1.0 / D)
                scr = temps.tile([128, D], F32, tag="scr")
                varsum = temps.tile([128, 1], F32, tag="varsum")
                nc.vector.scalar_tensor_tensor(
                    out=scr, in0=x, scalar=negmu[:, 0:1], in1=x,
                    op0=ALU.add, op1=ALU.mult, accum_out=varsum)
                stdv = temps.tile([128, 1], F32, tag="stdv")
                nc.scalar.activation(out=stdv, in_=varsum, func=AF.Sqrt,
                                     bias=eps_vec[:, 0:1], scale=1.0 / D)
                rstd = temps.tile([128, 1], F32, tag="rstd")
                nc.vector.reciprocal(rstd, stdv)
                scr2 = temps.tile([128, D], F32, tag="scr2")
                ssum = temps.tile([128, 1], F32, tag="ssum")
                nc.vector.scalar_tensor_tensor(
                    out=scr2, in0=x, scalar=negmu[:, 0:1], in1=w_ps,
                    op0=ALU.add, op1=ALU.mult, accum_out=ssum)
                nc.vector.tensor_mul(out=dst, in0=ssum, in1=rstd)

            def emit_sk_row():
                # transpose sk (128,NT_ALL) -> flat row in DRAM scratch, then
                # partition-broadcast it back to all 128 partitions.  Uses the
                # gpsimd (SWDGE) queue so the SP queue stays pure value DMAs.
                sk_dram = dscr.tile([NT_ALL, 128], F32, tag="sk_dram")
                nc.gpsimd.dma_start(out=sk_dram[:, :].transpose([1, 0]),
                                    in_=state["sk_colv"][:, :])
                sk_bc = consts.tile([128, N], F32, tag="sk_bc")
                sk_flat = sk_dram[:, :].rearrange("tt tp -> (tt tp)").unsqueeze(0)
                import concourse.bass as bass
                sk_bcast_src = bass.AP(
                    tensor=sk_flat.tensor, offset=sk_flat.offset,
                    ap=[[0, 128]] + list(sk_flat.ap)[1:])
                nc.gpsimd.dma_start(out=sk_bc, in_=sk_bcast_src)
                state["sk_bc"] = sk_bc

            def emit_softmax(k):
                logits = soft.tile([128, N], F32, tag="logits")
                nc.vector.tensor_scalar(
                    out=logits, in0=state["sk_bc"],
                    scalar1=state["sq_colv"][:, k:k + 1],
                    scalar2=state["bias_vec"][:, 0:1],
                    op0=ALU.add, op1=ALU.add)
                leak = soft.tile([128, N], F32, tag="leak")
                nc.vector.scalar_tensor_tensor(
                    out=leak, in0=logits, scalar=LEAKY_SLOPE, in1=logits,
                    op0=ALU.mult, op1=ALU.max)
                negmax = temps.tile([128, 1], F32, tag="negmax")
                nc.vector.tensor_reduce(out=negmax, in_=leak, axis=AX.X,
                                        op=ALU.max, negate=True)
                ex = soft.tile([128, N], F32, tag="ex")
                sume = temps.tile([128, 1], F32, tag="sume")
                nc.scalar.activation(out=ex, in_=leak, func=AF.Exp,
                                     bias=negmax[:, 0:1], scale=1.0,
                                     accum_out=sume[:, 0:1])
                rs = temps.tile([128, 1], F32, tag="rs")
                nc.vector.reciprocal(rs, sume)
                alph = soft.tile([128, N], F32, tag="alph")
                nc.vector.tensor_scalar_mul(out=alph, in0=ex, scalar1=rs[:, 0:1])
                nc.gpsimd.dma_start(out=alphas_out[k * 128:(k + 1) * 128, :],
                                    in_=alph)

            # ---------- interleaved emission ----------
            # Score-pipeline steps dripped between value groups so the DVE
            # never starves the output-DMA stream.
            # PE warm-up: ~3.5us of continuous small matmuls so the tensor
            # engine clock is at full speed when the first real broadcasts
            # arrive (PE ramps to 2.4GHz only after ~3us of sustained work).
            warm_ps = pbc.tile([128, 64], F32, tag="bc")
            NWARM = 16
            for w in range(NWARM):
                nc.tensor.matmul(warm_ps, lhsT=ones_col,
                                 rhs=ones_all[0:1, 0:64],
                                 start=(w == 0), stop=(w == NWARM - 1),
                                 skip_group_check=True)
            warm_scr = temps.tile([128, 1], F32, tag="warm_scr")
            nc.vector.tensor_reduce(out=warm_scr, in_=warm_ps, axis=AX.X,
                                    op=ALU.add)

            # body emitted `reps` times (reps>1 builds a timing NEFF whose
            # wall-clock slope isolates per-iteration HW time)
            for _rep in range(reps):
                emit_setup()
                steps = {
                    4: lambda: emit_ln_tile("all", 0),
                    6: lambda: emit_ln_tile("all", 1),
                    8: lambda: emit_ln_tile("all", 2),
                    10: lambda: emit_ln_tile("all", 3),
                    12: lambda: emit_ln_tile("own", 0),
                    14: lambda: emit_ln_tile("own", 1),
                    16: emit_sk_row,
                    24: lambda: emit_softmax(0),
                    28: lambda: emit_softmax(1),
                }
                segs = [2, 2, 4, 8] + [8] * 62
                j0 = 0
                for g, sz in enumerate(segs):
                    emit_value_seg(j0, sz)
                    j0 += sz
                    if g in steps:
                        steps[g]()
                state.clear()

    nc.compile()
    return nc


def _get_program():
    if "nc" not in _CACHE:
        _CACHE["nc"] = _build_program()
    return _CACHE["nc"]


def _make_in_maps(embeddings, att_w, att_b, ln_gamma, ln_beta):
    emb = np.ascontiguousarray(embeddings, dtype=np.float32)
    prm = np.concatenate([
        np.asarray(att_w, np.float32).reshape(-1),
        np.asarray(att_b, np.float32).reshape(-1),
        np.asarray(ln_gamma, np.float32).reshape(-1),
        np.asarray(ln_beta, np.float32).reshape(-1),
    ]).reshape(1, NPARAMS)
    in_maps = []
    for c in range(NCORES):
        b, i0 = c // 2, (c % 2) * NI
        eb = np.ascontiguousarray(emb[b])
        in_maps.append({
            "emb_all": eb,
            "emb_own": np.ascontiguousarray(emb[b, i0:i0 + NI]),
            "emb_flat": eb.reshape(1, N * D),
            "params": prm,
        })
    return in_maps


def kernel(embeddings, att_w, att_b, ln_gamma, ln_beta):
    from concourse.bass_utils import run_bass_kernel_spmd

    nc = _get_program()
    in_maps = _make_in_maps(embeddings, att_w, att_b, ln_gamma, ln_beta)
    res = run_bass_kernel_spmd(nc, in_maps, core_ids=list(range(NCORES)))
    _CACHE["last_results"] = res

    alphas = np.empty((B, N, N, 1), np.float32)
    value = np.empty((B, N, N, D), np.float32)
    for c in range(NCORES):
        b, i0 = c // 2, (c % 2) * NI
        r = res.results[c]
        alphas[b, i0:i0 + NI, :, 0] = r["alphas_out"]
        value[b, i0:i0 + NI] = r["value_out"]
    return alphas, value


# revision 32
# speedup vs baseline: 484.0284x; 5.3619x over previous
"""Trainium2 Bass kernel for nn_Att_cat_norm (gnn_message_passing).

reference:
    ln = LayerNorm(embeddings)                      # (B,N,D)
    sq = ln @ att_w[0,:D]; sk = ln @ att_w[0,D:]    # (B,N)
    logits = leaky_relu(sq[:,:,None] + sk[:,None,:] + att_b)
    alphas = softmax(logits, -1)[..., None]         # (B,N,N,1)
    value  = emb[:,:,None,:] * emb[:,None,:,:]      # (B,N,N,D)
    return alphas, value

Sharding: 8 cores; core c handles batch b=c//2, query rows i in
[i0, i0+256), i0=(c%2)*256.  Each core writes value rows (256,512,128)
= 64MB + alphas (256,512); output-DMA bound (~185us/core roofline).

Per-core device program (identical on all cores; per-core data arrives
via the inputs):
  value: partition dim = own query row (two blocks of 128).  Per
  segment of <=8 j-rows: PE broadcasts those rows to all 128
  partitions with an fp32 ones-weight matmul (verified bit-exact on
  HW: the fp32 two-pass split applies to the stationary operand, and
  1.0 splits exactly), then one DVE tensor_mul per i-block with a
  stride-0 broadcast view of the own-row tile, then one 0.5MB DMA
  whose descriptors are 4KB contiguous per partition.  Segment sizes
  ramp 2,2,4,8,8,... so the output-DMA stream starts early; a ~3.5us
  burst of tiny warm-up matmuls brings the PE clock to 2.4GHz before
  the first real broadcast.
  scores: LayerNorm stats via fused scalar_tensor_tensor + accum_out
  (var = E[(x-mu)x]); sq/sk as dot(x-mu, gamma*w)*rstd + dot(beta, w);
  sk transposed to a row via a DRAM-scratch write, then partition-
  broadcast back by DMA; logits = sk_bcast + sq + bias in one
  dual-scalar DVE op; leaky relu as one fused (0.01*x) max x DVE op;
  exp + row-sum in one ACT op; normalize on DVE.  The score pipeline
  runs on the gpsimd DMA queue and is dripped between early value
  segments so the SP value-DMA stream never starves while the serial
  LN chain runs.  TimelineSim: ~200us/core; DMA busy ~191us (the
  64MB value write at ~345GB/s is the roofline).
"""
import sys

if "/opt/trn_rl_repo" not in sys.path:
    sys.path.insert(0, "/opt/trn_rl_repo")

import numpy as np

B, N, D = 4, 512, 128
NI = N // 2          # own query rows per core
NCORES = 8
JG = 8               # j rows per value group
NJG = N // JG        # 64 value j-groups
NT_ALL = N // 128    # 4 token tiles of emb_all
NT_OWN = NI // 128   # 2 token tiles of emb_own
LEAKY_SLOPE = 0.01
LN_EPS = 1e-5
# params input layout: [att_w (2D) | att_b (1) | gamma (D) | beta (D)]
NPARAMS = 2 * D + 1 + D + D

_CACHE = {}


def _build_program(reps=1):
    import concourse.bass as bass
    import concourse.tile as tile
    from concourse import bacc, mybir

    F32 = mybir.dt.float32
    AF = mybir.ActivationFunctionType
    ALU = mybir.AluOpType
    AX = mybir.AxisListType

    nc = bacc.Bacc("TRN2", target_bir_lowering=False, debug=False,
                   num_devices=NCORES)

    emb_all = nc.dram_tensor("emb_all", (N, D), F32, kind="ExternalInput")
    emb_own = nc.dram_tensor("emb_own", (NI, D), F32, kind="ExternalInput")
    emb_flat = nc.dram_tensor("emb_flat", (1, N * D), F32, kind="ExternalInput")
    params = nc.dram_tensor("params", (1, NPARAMS), F32, kind="ExternalInput")

    value_out = nc.dram_tensor("value_out", (NI, N, D), F32, kind="ExternalOutput")
    alphas_out = nc.dram_tensor("alphas_out", (NI, N), F32, kind="ExternalOutput")

    with tile.TileContext(nc) as tc:
        with tc.tile_pool(name="consts", bufs=1) as consts, \
             tc.tile_pool(name="temps", bufs=3) as temps, \
             tc.tile_pool(name="soft", bufs=2) as soft, \
             tc.tile_pool(name="valp", bufs=10) as valp, \
             tc.tile_pool(name="pbc", bufs=4, space="PSUM") as pbc, \
             tc.tile_pool(name="dscr", bufs=1, space="DRAM") as dscr:

            # ---------- constants / inputs ----------
            ones_all = consts.tile([65, 128], F32)
            nc.vector.memset(ones_all, 1.0)
            ones_col = ones_all[0:1, :]
            eps_vec = consts.tile([128, 1], F32)
            nc.vector.memset(eps_vec, LN_EPS)

            # flat emb_all rows for the PE-broadcast rhs: 8-row groups
            # (4KB each) packed into partitions 0/32/64 (matmul base-
            # partition constraint), 22/22/20 groups per partition.
            prm = consts.tile([1, NPARAMS], F32)
            nc.sync.dma_start(out=prm, in_=params[:, :])
            emb_flat_sb = consts.tile([65, 22 * JG * D], F32)
            for pi, (g0, g1) in enumerate(((0, 22), (22, 44), (44, 64))):
                nc.sync.dma_start(
                    out=emb_flat_sb[32 * pi:32 * pi + 1, 0:(g1 - g0) * JG * D],
                    in_=emb_flat[0:1, g0 * JG * D:g1 * JG * D])
            emb_own_t = consts.tile([128, NT_OWN, D], F32)
            nc.sync.dma_start(
                out=emb_own_t,
                in_=emb_own[:, :].rearrange("(tt tp) d -> tp tt d", tp=128))
            emb_all_t = consts.tile([128, NT_ALL, D], F32)
            nc.sync.dma_start(
                out=emb_all_t,
                in_=emb_all[:, :].rearrange("(tt tp) d -> tp tt d", tp=128))
            w_q = prm[:, 0:D]
            w_k = prm[:, D:2 * D]
            att_b = prm[:, 2 * D:2 * D + 1]
            gamma = prm[:, 2 * D + 1:3 * D + 1]
            beta = prm[:, 3 * D + 1:4 * D + 1]

            # ---------- value segment emitter ----------
            # j rows [j0, j0+nrows): PE-broadcast them to all partitions,
            # multiply with the own-row tile, DMA out.  nrows <= JG.
            def emit_value_seg(j0, nrows):
                part = 32 * (j0 // 176)
                off = (j0 - 176 * (j0 // 176)) * D
                bc_ps = pbc.tile([128, JG * D], F32, tag="bc")
                nmm = (nrows * D + 511) // 512
                for q in range(nmm):
                    f0, f1 = q * 512, min((q + 1) * 512, nrows * D)
                    nc.tensor.matmul(
                        bc_ps[:, f0:f1],
                        lhsT=ones_all[part:part + 1, :],
                        rhs=emb_flat_sb[part:part + 1, off + f0:off + f1],
                        start=True, stop=True)
                for k in range(NT_OWN):
                    val = valp.tile([128, JG * D], F32, tag="val")
                    in0 = emb_own_t[:, k, :].unsqueeze(1).broadcast_to(
                        (128, nrows, D))
                    in1 = bc_ps[:, 0:nrows * D].rearrange(
                        "p (j d) -> p j d", j=nrows)
                    outv = val[:, 0:nrows * D].rearrange(
                        "p (j d) -> p j d", j=nrows)
                    nc.vector.tensor_mul(out=outv, in0=in0, in1=in1)
                    nc.sync.dma_start(
                        out=value_out[k * 128:(k + 1) * 128,
                                      j0:j0 + nrows, :],
                        in_=outv)

            # ---------- score-pipeline emitters ----------
            state = {}

            def emit_setup():
                # wq' = gamma*wq, wk' = gamma*wk; bias = att_b+beta.wq+beta.wk
                wqp = consts.tile([1, D], F32, tag="wqp")
                nc.vector.tensor_mul(out=wqp, in0=w_q, in1=gamma)
                wkp = consts.tile([1, D], F32, tag="wkp")
                nc.vector.tensor_mul(out=wkp, in0=w_k, in1=gamma)
                scrq = temps.tile([1, D], F32, tag="scr1")
                cq = consts.tile([1, 1], F32, tag="cq")
                nc.vector.scalar_tensor_tensor(
                    out=scrq, in0=w_q, scalar=0.0, in1=beta,
                    op0=ALU.add, op1=ALU.mult, accum_out=cq)
                scrk = temps.tile([1, D], F32, tag="scr1")
                ck = consts.tile([1, 1], F32, tag="ck")
                nc.vector.scalar_tensor_tensor(
                    out=scrk, in0=w_k, scalar=0.0, in1=beta,
                    op0=ALU.add, op1=ALU.mult, accum_out=ck)
                bias_tot = consts.tile([1, 1], F32, tag="bias_tot")
                nc.vector.tensor_add(out=bias_tot, in0=cq, in1=ck)
                nc.vector.tensor_add(out=bias_tot, in0=bias_tot, in1=att_b)

                bvec_ps = pbc.tile([128, 1], F32, tag="bc")
                nc.tensor.matmul(bvec_ps, lhsT=ones_col, rhs=bias_tot,
                                 start=True, stop=True)
                bias_vec = consts.tile([128, 1], F32, tag="bias_vec")
                nc.scalar.copy(out=bias_vec, in_=bvec_ps)
                wqb_ps = pbc.tile([128, D], F32, tag="bc")
                nc.tensor.matmul(wqb_ps, lhsT=ones_col, rhs=wqp,
                                 start=True, stop=True)
                wqb = consts.tile([128, D], F32, tag="wqb")
                nc.scalar.copy(out=wqb, in_=wqb_ps)
                wkb_ps = pbc.tile([128, D], F32, tag="bc")
                nc.tensor.matmul(wkb_ps, lhsT=ones_col, rhs=wkp,
                                 start=True, stop=True)
                wkb = consts.tile([128, D], F32, tag="wkb")
                nc.scalar.copy(out=wkb, in_=wkb_ps)
                state.update(bias_vec=bias_vec, wqb=wqb, wkb=wkb)
                state["sq_colv"] = consts.tile([128, NT_OWN], F32, tag="sq_colv", name="sq_colv")
                state["sk_colv"] = consts.tile([128, NT_ALL], F32, tag="sk_colv", name="sk_colv")

            def emit_ln_tile(which, tt):
                # LayerNorm stats + raw q/k score for one 128-token tile.
                if which == "all":
                    x, w_ps, dst = (emb_all_t[:, tt, :], state["wkb"],
                                    state["sk_colv"][:, tt:tt + 1])
                else:
                    x, w_ps, dst = (emb_own_t[:, tt, :], state["wqb"],
                                    state["sq_colv"][:, tt:tt + 1])
                musum = temps.tile([128, 1], F32, tag="musum")
                nc.vector.tensor_reduce(out=musum, in_=x, axis=AX.X, op=ALU.add)
                negmu = temps.tile([128, 1], F32, tag="negmu")
                nc.vector.tensor_scalar_mul(out=negmu, in0=musum,
                                            scalar1=-1.0 / D)
                scr = temps.tile([128, D], F32, tag="scr")
                varsum = temps.tile([128, 1], F32, tag="varsum")
                nc.vector.scalar_tensor_tensor(
                    out=scr, in0=x, scalar=negmu[:, 0:1], in1=x,
                    op0=ALU.add, op1=ALU.mult, accum_out=varsum)
                stdv = temps.tile([128, 1], F32, tag="stdv")
                nc.scalar.activation(out=stdv, in_=varsum, func=AF.Sqrt,
                                     bias=eps_vec[:, 0:1], scale=1.0 / D)
                rstd = temps.tile([128, 1], F32, tag="rstd")
                nc.vector.reciprocal(rstd, stdv)
                scr2 = temps.tile([128, D], F32, tag="scr2")
                ssum = temps.tile([128, 1], F32, tag="ssum")
                nc.vector.scalar_tensor_tensor(
                    out=scr2, in0=x, scalar=negmu[:, 0:1], in1=w_ps,
                    op0=ALU.add, op1=ALU.mult, accum_out=ssum)
                nc.vector.tensor_mul(out=dst, in0=ssum, in1=rstd)

            def emit_sk_row():
                # transpose sk (128,NT_ALL) -> flat row in DRAM scratch, then
                # partition-broadcast it back to all 128 partitions.  Uses the
                # gpsimd (SWDGE) queue so the SP queue stays pure value DMAs.
                sk_dram = dscr.tile([NT_ALL, 128], F32, tag="sk_dram")
                nc.gpsimd.dma_start(out=sk_dram[:, :].transpose([1, 0]),
                                    in_=state["sk_colv"][:, :])
                sk_bc = consts.tile([128, N], F32, tag="sk_bc")
                sk_flat = sk_dram[:, :].rearrange("tt tp -> (tt tp)").unsqueeze(0)
                sk_bcast_src = bass.AP(
                    tensor=sk_flat.tensor, offset=sk_flat.offset,
                    ap=[[0, 128]] + list(sk_flat.ap)[1:])
                nc.gpsimd.dma_start(out=sk_bc, in_=sk_bcast_src)
                state["sk_bc"] = sk_bc

            def emit_softmax(k):
                logits = soft.tile([128, N], F32, tag="logits")
                nc.vector.tensor_scalar(
                    out=logits, in0=state["sk_bc"],
                    scalar1=state["sq_colv"][:, k:k + 1],
                    scalar2=state["bias_vec"][:, 0:1],
                    op0=ALU.add, op1=ALU.add)
                leak = soft.tile([128, N], F32, tag="leak")
                nc.vector.scalar_tensor_tensor(
                    out=leak, in0=logits, scalar=LEAKY_SLOPE, in1=logits,
                    op0=ALU.mult, op1=ALU.max)
                negmax = temps.tile([128, 1], F32, tag="negmax")
                nc.vector.tensor_reduce(out=negmax, in_=leak, axis=AX.X,
                                        op=ALU.max, negate=True)
                ex = soft.tile([128, N], F32, tag="ex")
                sume = temps.tile([128, 1], F32, tag="sume")
                nc.scalar.activation(out=ex, in_=leak, func=AF.Exp,
                                     bias=negmax[:, 0:1], scale=1.0,
                                     accum_out=sume[:, 0:1])
                rs = temps.tile([128, 1], F32, tag="rs")
                nc.vector.reciprocal(rs, sume)
                alph = soft.tile([128, N], F32, tag="alph")
                nc.vector.tensor_scalar_mul(out=alph, in0=ex, scalar1=rs[:, 0:1])
                nc.gpsimd.dma_start(out=alphas_out[k * 128:(k + 1) * 128, :],
                                    in_=alph)

            # ---------- interleaved emission ----------
            # Score-pipeline steps dripped between value groups so the DVE
            # never starves the output-DMA stream.
            # PE warm-up: ~3.5us of continuous small matmuls so the tensor
            # engine clock is at full speed when the first real broadcasts
            # arrive (PE ramps to 2.4GHz only after ~3us of sustained work).
            warm_ps = pbc.tile([128, 64], F32, tag="bc")
            NWARM = 16
            for w in range(NWARM):
                nc.tensor.matmul(warm_ps, lhsT=ones_col,
                                 rhs=ones_all[0:1, 0:64],
                                 start=(w == 0), stop=(w == NWARM - 1),
                                 skip_group_check=True)
            warm_scr = temps.tile([128, 1], F32, tag="warm_scr")
            nc.vector.tensor_reduce(out=warm_scr, in_=warm_ps, axis=AX.X,
                                    op=ALU.add)

            # body emitted `reps` times (reps>1 builds a timing NEFF whose
            # wall-clock slope isolates per-iteration HW time)
            for _rep in range(reps):
                emit_setup()
                steps = {
                    4: lambda: emit_ln_tile("all", 0),
                    6: lambda: emit_ln_tile("all", 1),
                    8: lambda: emit_ln_tile("all", 2),
                    10: lambda: emit_ln_tile("all", 3),
                    12: lambda: emit_ln_tile("own", 0),
                    14: lambda: emit_ln_tile("own", 1),
                    16: emit_sk_row,
                    24: lambda: emit_softmax(0),
                    28: lambda: emit_softmax(1),
                }
                segs = [2, 2, 4, 8] + [8] * 62
                j0 = 0
                for g, sz in enumerate(segs):
                    emit_value_seg(j0, sz)
                    j0 += sz
                    if g in steps:
                        steps[g]()
                state.clear()

    nc.compile()
    return nc


def _get_program():
    if "nc" not in _CACHE:
        _CACHE["nc"] = _build_program()
    return _CACHE["nc"]


def _make_in_maps(embeddings, att_w, att_b, ln_gamma, ln_beta):
    emb = np.ascontiguousarray(embeddings, dtype=np.float32)
    prm = np.concatenate([
        np.asarray(att_w, np.float32).reshape(-1),
        np.asarray(att_b, np.float32).reshape(-1),
        np.asarray(ln_gamma, np.float32).reshape(-1),
        np.asarray(ln_beta, np.float32).reshape(-1),
    ]).reshape(1, NPARAMS)
    in_maps = []
    for c in range(NCORES):
        b, i0 = c // 2, (c % 2) * NI
        eb = np.ascontiguousarray(emb[b])
        in_maps.append({
            "emb_all": eb,
            "emb_own": np.ascontiguousarray(emb[b, i0:i0 + NI]),
            "emb_flat": eb.reshape(1, N * D),
            "params": prm,
        })
    return in_maps


def kernel(embeddings, att_w, att_b, ln_gamma, ln_beta):
    from concourse.bass_utils import run_bass_kernel_spmd

    nc = _get_program()
    in_maps = _make_in_maps(embeddings, att_w, att_b, ln_gamma, ln_beta)
    res = run_bass_kernel_spmd(nc, in_maps, core_ids=list(range(NCORES)))
    _CACHE["last_results"] = res

    alphas = np.empty((B, N, N, 1), np.float32)
    value = np.empty((B, N, N, D), np.float32)
    for c in range(NCORES):
        b, i0 = c // 2, (c % 2) * NI
        r = res.results[c]
        alphas[b, i0:i0 + NI, :, 0] = r["alphas_out"]
        value[b, i0:i0 + NI] = r["value_out"]
    return alphas, value
